# revision 27
# baseline (speedup 1.0000x reference)
"""DopplerPTNet point-transformer block on 8 Trainium2 NeuronCores.

Strategy (point-parallel, per the sharding hint):
  - Shard the N points across 8 cores (N/8 each); replicate small weights.
  - Pre-pass per core: build its shard of a packed bf16 "kv table" with one
    1040-byte row per point: [ k(256) | v(256) | tg(3) | pad(5) ], where k is
    pre-scaled by bn1 gamma/rsqrt(var), tg = A1@xyz + c1 is the position
    encoder's first affine, plus a separate [N,1] f32 vh table (velocity
    encoding scalar, post BN+ReLU).  AllGather both tables so every core has
    the full N-row tables in local DRAM.
  - Main pass per 128-point tile: one 2048-index indirect DMA gathers kv rows
    point-major into SBUF; an SBUF-source dma_gather with a constant int16
    identity permutation transposes the k half to channel-major [256, 2048];
    the attention-logit MLP runs channel-major (PE matmuls, BN folded into
    weights and per-partition ACT biases); per-neighbor W2 matmuls emit
    point-major logits; softmax and the share-grouped weighted aggregation
    run point-major with DVE tree reductions; residual + rho close the tile.

All BatchNorms are inference-affine and folded on the host.  w_b2 is dropped
(softmax-invariant); bv+p_b2+v_b2 is absorbed into r_bn's mean because
sum(attn)==1.
"""

import sys

sys.path.insert(0, "/opt/trn_rl_repo")

import numpy as np
import ml_dtypes

import concourse.bass as bass
import concourse.mybir as mybir
import concourse.tile as tile
from concourse import bacc
from concourse import bass_isa
from concourse.bass import IndirectOffsetOnAxis
from concourse.bass_utils import run_bass_kernel_spmd
from concourse.masks import make_identity
import concourse.tile_sem_assignment as tsa

# Partition the 8 DMASW completion lanes by SWDGE queue so multi-queue
# descriptor generation (different Q7 core pairs) doesn't share sem lanes.
_QLANES = {0: (0, 1, 2, 3, 4, 5), 1: (6,), 2: (7,)}
_ORIG_ASSIGN_TICK = tsa.TileClockTick._assign_tick


def _assign_tick_qaware(self, inst):
    if (
        isinstance(inst, tsa.DMAInst)
        and inst.engine == mybir.EngineType.Pool
        and not isinstance(inst, bass_isa.UserSyncedRemoteDMADescs)
    ):
        qn = getattr(inst, "queue_num", 0) or 0
        lanes = _QLANES.get(qn, _QLANES[0])
        rr = getattr(self, "_swdge_q_rr", None)
        if rr is None:
            rr = self._swdge_q_rr = {}
        i = rr.get(qn, 0)
        rr[qn] = i + 1
        save = self.next_sw_dma_idx
        self.next_sw_dma_idx = lanes[i % len(lanes)]
        try:
            return _ORIG_ASSIGN_TICK(self, inst)
        finally:
            self.next_sw_dma_idx = save
    return _ORIG_ASSIGN_TICK(self, inst)


tsa.TileClockTick._assign_tick = _assign_tick_qaware

BF16 = mybir.dt.bfloat16
F32 = mybir.dt.float32
I32 = mybir.dt.int32
I16 = mybir.dt.int16
AOP = mybir.AluOpType
AFT = mybir.ActivationFunctionType

NCORES = 8
C = 256
NS = 16
CS = 32
S = 8
P = 128
ROW = 520          # bf16 elems per kv row: k 256 | v 256 | tg 3 | pad 5
ROW_B = ROW * 2    # 1040 bytes
EPS = 1e-5
NE = NS * P        # edges per tile = 2048


def _bf(x):
    return np.ascontiguousarray(np.asarray(x, dtype=np.float32).astype(ml_dtypes.bfloat16))


def _f32(x):
    return np.ascontiguousarray(x, dtype=np.float32)


def build_program(n_total: int, debug_taps: bool = False):
    """Build the SPMD bass program (identical on all 8 cores)."""
    npc = n_total // NCORES          # points per core
    nt = npc // P                    # 128-point tiles per core

    nc = bacc.Bacc(
        "TRN2",
        target_bir_lowering=False,
        debug=False,
        enable_asserts=False,
        num_devices=NCORES,
        num_swdge_queues=3,
        dynamic_dma_scratch_size=24576,
    )

    # ---- I/O -----------------------------------------------------------
    def inp(name, shape, dt):
        return nc.dram_tensor(name, shape, dt, kind="ExternalInput")

    feats_sh = inp("feats_sh", [npc, C], F32)
    xyz_sh = inp("xyz_sh", [npc, 3], F32)
    vel_sh = inp("vel_sh", [npc, 1], F32)
    idx_sh = inp("idx_sh", [npc, NS], I32)
    idxvf_sh = inp("idxvf_sh", [npc // P, 2, P, NE // 32], I16)
    vsel_sh = inp("vsel_sh", [npc, NS], F32)
    viota_c = inp("viota_c", [1, NS * 64], F32)

    wq_t = inp("wq_t", [2, P, C], BF16)        # (scale1*Wq).T cin-groups
    wkv_t = inp("wkv_t", [2, P, 2 * C], BF16)  # [(scale1*Wk).T | Wv.T]
    w2cat_w = inp("w2cat_w", [4, C], BF16)     # pe lhsT (scaled by scale1)
    w2cat_v = inp("w2cat_v", [4, C], BF16)     # pe rhs for vals (unscaled)
    w1_t = inp("w1_t", [2, P, CS], BF16)       # (scale2*w_w1).T
    w2_t = inp("w2_t", [CS, CS], BF16)         # w_w2.T
    rw2 = inp("rw2", [2, P, C], BF16)          # r_w.T
    a1_t = inp("a1_t", [4, 4], F32)            # A1.T padded
    shift1_c = inp("shift1_c", [P, 2], F32)    # bn1 shift per channel
    shift2_c = inp("shift2_c", [CS, 1], F32)   # bn2 shift per cs
    svbv_c = inp("svbv_c", [1, 2], F32)        # velocity scale/bias
    c1_c = inp("c1_c", [1, 4], F32)            # pos affine bias
    scaler_c = inp("scaler_c", [1, C], BF16)   # rho bn scale
    shiftr_c = inp("shiftr_c", [1, C], BF16)   # rho bn shift
    rb_c = inp("rb_c", [1, C], F32)            # r_b

    out_ext = nc.dram_tensor("out", [npc, C], F32, kind="ExternalOutput")
    taps = {}
    if debug_taps:
        for nm, shp, dt in [
            ("tap_stag", [P, NS * ROW], BF16),
            ("tap_pmx", [P, NS * 4], F32),
            ("tap_T2", [4, NE], BF16),
            ("tap_kcm", [P, 2 * NE], BF16),
            ("tap_q", [P, 2 * P], BF16),
            ("tap_t1", [P, 2 * NE], BF16),
            ("tap_t2", [CS, NE], BF16),
            ("tap_attnE", [P, NS * CS], BF16),
            ("tap_ssum", [P, CS], F32),
            ("tap_vals", [P, NS * C], BF16),
            ("tap_agg", [P, C], BF16),
            ("tap_rin", [P, C], BF16),
        ]:
            taps[nm] = nc.dram_tensor(nm, shp, dt, kind="ExternalOutput")

    def tap(t, nm, ap):
        if debug_taps and t == 0:
            nc.sync.dma_start(out=taps[nm][:, :], in_=ap)

    # ---- internal DRAM -------------------------------------------------
    featsb = nc.dram_tensor("featsb", [npc, C], BF16)
    kv_shard = nc.dram_tensor("kv_shard", [npc, ROW], BF16)
    kv_full = nc.dram_tensor("kv_full", [n_total, ROW], BF16, addr_space="Shared")
    vh_shard = nc.dram_tensor("vh_shard", [npc, 1], F32)
    vh_full = nc.dram_tensor("vh_full", [n_total, 1], F32, addr_space="Shared")

    rg = [list(range(NCORES))]

    with tile.TileContext(nc) as tc:
        with (
            tc.tile_pool(name="const", bufs=1) as cpool,
            tc.tile_pool(name="work", bufs=2) as pool,
            tc.tile_pool(name="big", bufs=2) as bigpool,
            tc.tile_pool(name="stagp", bufs=3) as stagpool,
            tc.tile_pool(name="ps_a", bufs=2, space="PSUM") as ps_a,
            tc.tile_pool(name="ps_b", bufs=2, space="PSUM") as ps_b,
        ):
            # ---------- constants ----------
            ident_b = cpool.tile([P, P], BF16, tag="ident_b")
            make_identity(nc, ident_b[:])
            ident_f = cpool.tile([P, P], F32, tag="ident_f")
            make_identity(nc, ident_f[:])
            def cload(src, shape, dt, tag):
                t = cpool.tile(shape, dt, tag=tag)
                nc.sync.dma_start(out=t[:], in_=src)
                return t

            # group-split weights as [P, 2, X] (partition-first)
            def gload(src, width, tag):
                t = cpool.tile([P, 2, width], BF16, tag=tag)
                for g in range(2):
                    nc.sync.dma_start(out=t[:, g, :], in_=src[g, :, :])
                return t

            wq_sb = gload(wq_t, C, "wq")
            wkv_sb = gload(wkv_t, 2 * C, "wkv")
            w1_sb = gload(w1_t, CS, "w1")
            rw2_sb = gload(rw2, C, "rw2")
            w2w_sb = cload(w2cat_w[:, :], [4, C], BF16, "w2w")
            w2v_sb = cload(w2cat_v[:, :], [4, C], BF16, "w2v")
            w2_sb = cload(w2_t[:, :], [CS, CS], BF16, "w2")
            a1_sb = cload(a1_t[:, :], [4, 4], F32, "a1")
            sh1_sb = cload(shift1_c[:, :], [P, 2], F32, "sh1")
            sh2_sb = cload(shift2_c[:, :], [CS, 1], F32, "sh2")

            def bcast(name, src, width, dt):
                row = cpool.tile([1, width], dt, tag=name + "r")
                nc.sync.dma_start(out=row[:], in_=src)
                full = cpool.tile([P, width], dt, tag=name)
                nc.gpsimd.partition_broadcast(full[:], row[:])
                return full

            svbv_sb = bcast("svbv", svbv_c[:, :], 2, F32)
            viota_sb = bcast("viota", viota_c[:, :], NS * 64, F32)
            c1_sb = bcast("c1", c1_c[:, :], 4, F32)
            sclr_sb = bcast("sclr", scaler_c[:, :], C, BF16)
            shfr_sb = bcast("shfr", shiftr_c[:, :], C, BF16)
            rb_sb = bcast("rb", rb_c[:, :], C, F32)

            # ---------- phase A: feats -> bf16 copy ----------
            rows_per = min(npc, 1024)
            cast_cols = rows_per * C // P
            for ch in range(npc // rows_per):
                sl = slice(ch * rows_per, (ch + 1) * rows_per)
                cb = bigpool.tile([P, cast_cols], BF16, tag="castbuf")
                nc.gpsimd.dma_start(
                    out=cb[:],
                    in_=feats_sh[sl, :].rearrange("(p j) c -> p (j c)", p=P),
                )
                nc.sync.dma_start(
                    out=featsb[sl, :].rearrange("(p j) c -> p (j c)", p=P),
                    in_=cb[:],
                )

            # ---------- phase B: kv/vh table shard ----------
            vh_acc = cpool.tile([P, nt], F32, tag="vh_acc")
            for t in range(nt):
                rsl = slice(t * P, (t + 1) * P)
                ftT = pool.tile([P, 2, P], BF16, tag="ftT")
                for g in range(2):
                    nc.sync.dma_start(
                        out=ftT[:, g, :],
                        in_=featsb[rsl, g * P : (g + 1) * P],
                        transpose=True,
                    )
                kv_ps = ps_a.tile([P, 2 * C], F32, tag="pa")
                for g in range(2):
                    nc.tensor.matmul(
                        out=kv_ps[:],
                        lhsT=ftT[:, g, :],
                        rhs=wkv_sb[:, g, :],
                        start=(g == 0),
                        stop=(g == 1),
                    )
                row_t = pool.tile([P, ROW], BF16, tag="row_t")
                nc.vector.tensor_copy(row_t[:, 0 : 2 * C], kv_ps[:])

                xyz_t = pool.tile([P, 3], F32, tag="xyz_t")
                nc.sync.dma_start(out=xyz_t[:], in_=xyz_sh[rsl, :])
                xT_ps = ps_b.tile([P, P], F32, tag="pb")
                nc.tensor.transpose(
                    out=xT_ps[:3, :], in_=xyz_t[:], identity=ident_f[:]
                )
                xT_sb = pool.tile([4, P], F32, tag="xT_sb")
                nc.vector.tensor_copy(xT_sb[:3, :], xT_ps[:3, :])
                tg_ps = ps_b.tile([P, P], F32, tag="pb")
                nc.tensor.matmul(
                    out=tg_ps[:, :3],
                    lhsT=xT_sb[:3, :],
                    rhs=a1_sb[:3, :3],
                    start=True,
                    stop=True,
                )
                nc.vector.scalar_tensor_tensor(
                    out=row_t[:, 2 * C : 2 * C + 3],
                    in0=tg_ps[:, :3],
                    scalar=1.0,
                    in1=c1_sb[:, :3],
                    op0=AOP.mult,
                    op1=AOP.add,
                )
                nc.vector.memset(row_t[:, 2 * C + 3 : ROW], 0)
                vel_t = pool.tile([P, 1], F32, tag="vel_t")
                nc.sync.dma_start(out=vel_t[:], in_=vel_sh[rsl, :])
                nc.scalar.activation(
                    vh_acc[:, t : t + 1],
                    vel_t[:],
                    AFT.Relu,
                    bias=svbv_sb[:, 1:2],
                    scale=svbv_sb[:, 0:1],
                )
                nc.sync.dma_start(out=kv_shard[rsl, :], in_=row_t[:])

            # vh shard: transpose [p, t] -> point order, write once
            vhT_ps = ps_a.tile([P, P], F32, tag="pa")
            nc.tensor.transpose(
                out=vhT_ps[:nt, :], in_=vh_acc[:], identity=ident_f[:]
            )
            vhT_sb = pool.tile([P, P], F32, tag="vhT_sb")
            nc.vector.tensor_copy(vhT_sb[:nt, :], vhT_ps[:nt, :])
            nc.sync.dma_start(
                out=vh_shard[:, :].rearrange("(t p) o -> t (p o)", p=P),
                in_=vhT_sb[:nt, :],
            )

            # ---------- phase C: all-gather tables ----------
            nc.gpsimd.collective_compute(
                "AllGather",
                AOP.bypass,
                replica_groups=rg,
                ins=[kv_shard.ap().opt()],
                outs=[kv_full.ap().opt()],
            )
            nc.gpsimd.collective_compute(
                "AllGather",
                AOP.bypass,
                replica_groups=rg,
                ins=[vh_shard.ap().opt()],
                outs=[vh_full.ap().opt()],
            )

            # ---------- phase D: main pass ----------
            for t in range(nt):
                rsl = slice(t * P, (t + 1) * P)

                idx_t = pool.tile([P, NS], I32, tag="idx_t")
                nc.sync.dma_start(out=idx_t[:], in_=idx_sh[rsl, :])

                # stage-1: gather kv rows point-major [128][16][520]
                stag = stagpool.tile([P, NS * ROW], BF16, tag="stag")
                stag3 = stag[:, :].rearrange("p (s e) -> p s e", s=NS)
                for s in range(NS):
                    nc.gpsimd.indirect_dma_start(
                        out=stag3[:, s, :],
                        out_offset=None,
                        in_=kv_full[:, :],
                        in_offset=IndirectOffsetOnAxis(
                            ap=idx_t[:, s : s + 1], axis=0
                        ),
                    )
                # velocity: gather 64-wide fat rows, one-hot select.
                # Split into two 1024-idx halves on queues 1/2 so both Q7
                # core pairs generate descriptors concurrently.
                velf = bigpool.tile([P, NS * 64], F32, tag="velf")
                for h in range(2):
                    idxvf_t = pool.tile([P, NE // 32], I16, tag=f"idxvf{h}")
                    nc.sync.dma_start(
                        out=idxvf_t[:], in_=idxvf_sh[t, h, :, :]
                    )
                    nc.gpsimd.dma_gather(
                        out_ap=velf[
                            :, h * (NE // 4) : (h + 1) * (NE // 4)
                        ].rearrange("p (s k) -> p s k", k=64),
                        in_ap=vh_full[:, :].rearrange(
                            "(m k) o -> m (k o)", k=64
                        ),
                        idxs_ap=idxvf_t[:, :],
                        num_idxs=NE // 2,
                        num_idxs_reg=NE // 2,
                        elem_size=64,
                        transpose=False,
                        single_packet=False,
                        queue_num=1 + h,
                    )
                vsel_t = pool.tile([P, NS], F32, tag="vsel_t")
                nc.sync.dma_start(out=vsel_t[:], in_=vsel_sh[rsl, :])
                vmask = bigpool.tile([P, NS * 64], BF16, tag="vmask")
                nc.vector.tensor_tensor(
                    out=vmask[:].rearrange("p (s k) -> p s k", k=64),
                    in0=viota_sb[:, :].rearrange("p (s k) -> p s k", k=64),
                    in1=vsel_t[:, :]
                    .rearrange("p (s o) -> p s o", o=1)
                    .to_broadcast([P, NS, 64]),
                    op=AOP.is_equal,
                )
                nc.vector.tensor_tensor(
                    out=vmask[:], in0=vmask[:], in1=velf[:], op=AOP.mult
                )
                vtr = pool.tile([P, NS * 62], BF16, tag="vtr")
                vw = 64
                cur_v = vmask[:, :]
                off_v = 0
                while vw > 2:
                    half = vw // 2
                    dst_v = vtr[:, off_v : off_v + NS * half]
                    nc.vector.tensor_tensor(
                        out=dst_v.rearrange("p (s k) -> p s k", k=half),
                        in0=cur_v.rearrange("p (s k) -> p s k", k=vw)[
                            :, :, 0:vw:2
                        ],
                        in1=cur_v.rearrange("p (s k) -> p s k", k=vw)[
                            :, :, 1:vw:2
                        ],
                        op=AOP.add,
                    )
                    cur_v = dst_v
                    off_v += NS * half
                    vw = half
                # feats tiles
                ftT = pool.tile([P, 2, P], BF16, tag="ftT")
                for g in range(2):
                    nc.sync.dma_start(
                        out=ftT[:, g, :],
                        in_=featsb[rsl, g * P : (g + 1) * P],
                        transpose=True,
                    )
                feats_pm = pool.tile([P, C], BF16, tag="feats_pm")
                nc.sync.dma_start(out=feats_pm[:], in_=featsb[rsl, :])

                # q point-major (scaled by scale1), subtracted into stag's
                # k half so stage-2 transposes (k - q) directly
                q_ps = ps_b.tile([P, C], F32, tag="pb")
                for g in range(2):
                    nc.tensor.matmul(
                        out=q_ps[:],
                        lhsT=ftT[:, g, :],
                        rhs=wq_sb[:, g, :],
                        start=(g == 0),
                        stop=(g == 1),
                    )
                q_pm = pool.tile([P, C], BF16, tag="q_pm")
                nc.scalar.copy(q_pm[:], q_ps[:])
                nc.vector.tensor_tensor(
                    out=stag3[:, :, 0:C],
                    in0=stag3[:, :, 0:C],
                    in1=q_pm[:, :]
                    .rearrange("p (o c) -> p o c", o=1)
                    .to_broadcast([P, NS, C]),
                    op=AOP.subtract,
                )

                # stage-2: (k - q) half -> channel-major [128, 2, 2048]
                # via PE transposes of 128x128 blocks (no Q7 involvement)
                k_cm = bigpool.tile([P, 2, NE], BF16, tag="k_cm")
                for cg in range(2):
                    for g in range(4):
                        kT_ps = ps_b.tile([P, 4 * P], BF16, tag="pbk")
                        for j in range(4):
                            ns = g * 4 + j
                            nc.tensor.transpose(
                                out=kT_ps[:, j * P : (j + 1) * P],
                                in_=stag3[:, ns, cg * P : (cg + 1) * P],
                                identity=ident_b[:],
                            )
                        nc.scalar.copy(
                            k_cm[:, cg, g * 4 * P : (g + 1) * 4 * P],
                            kT_ps[:],
                        )

                tap(t, "tap_stag", stag[:, :])

                # pos/vel encoding, point-major [128, (ns,4)] then transpose
                xyz_t = pool.tile([P, 3], F32, tag="xyz_t")
                nc.sync.dma_start(out=xyz_t[:], in_=xyz_sh[rsl, :])
                xT_ps = ps_b.tile([P, P], F32, tag="pb")
                nc.tensor.transpose(
                    out=xT_ps[:3, :], in_=xyz_t[:], identity=ident_f[:]
                )
                xT_sb = pool.tile([4, P], F32, tag="xT_sb")
                nc.vector.tensor_copy(xT_sb[:3, :], xT_ps[:3, :])
                axc_ps = ps_b.tile([P, P], F32, tag="pb")
                nc.tensor.matmul(
                    out=axc_ps[:, :3],
                    lhsT=xT_sb[:3, :],
                    rhs=a1_sb[:3, :3],
                    start=True,
                    stop=True,
                )
                pmx = pool.tile([P, NS, 4], BF16, tag="pmx")
                nc.vector.scalar_tensor_tensor(
                    out=pmx[:, :, 0:3],
                    in0=stag3[:, :, 2 * C : 2 * C + 3],
                    scalar=1.0,
                    in1=axc_ps[:, :3]
                    .rearrange("p (o d) -> p o d", o=1)
                    .to_broadcast([P, NS, 3]),
                    op0=AOP.mult,
                    op1=AOP.subtract,
                )
                nc.vector.tensor_scalar_max(pmx[:, :, 0:3], pmx[:, :, 0:3], 0.0)
                nc.vector.tensor_tensor(
                    out=pmx[:, :, 3:4],
                    in0=cur_v.rearrange("p (s k) -> p s k", k=2)[:, :, 0:1],
                    in1=cur_v.rearrange("p (s k) -> p s k", k=2)[:, :, 1:2],
                    op=AOP.add,
                )
                tap(t, "tap_pmx", pmx[:, :, :].rearrange("p s d -> p (s d)"))
                # per-ns transposes -> T2 [4, (ns, n)] channel-major
                T2_sb = pool.tile([4, NE], BF16, tag="T2_sb")
                for g in range(4):
                    T2_ps = ps_b.tile([4, 4 * P], BF16, tag="pbt")
                    for j in range(4):
                        ns = g * 4 + j
                        nc.tensor.transpose(
                            out=T2_ps[:, j * P : (j + 1) * P],
                            in_=pmx[:, ns, :],
                            identity=ident_b[:],
                        )
                    nc.scalar.copy(
                        T2_sb[:, g * 4 * P : (g + 1) * 4 * P], T2_ps[:]
                    )

                tap(t, "tap_T2", T2_sb[:, :])
                tap(t, "tap_kcm", k_cm[:, :, :].rearrange("p e n -> p (e n)"))
                tap(t, "tap_q", q_pm[:, :])
                # ---- logits pipeline: w = (k - q) + pe, bn1+relu ----
                t1 = bigpool.tile([P, 2, NE], BF16, tag="t1")
                for cg in range(2):
                    for g in range(4):
                        w_ps = ps_a.tile([P, 4 * P], F32, tag="pa")
                        nc.tensor.matmul(
                            out=w_ps[:],
                            lhsT=w2w_sb[:, cg * P : (cg + 1) * P],
                            rhs=T2_sb[:, g * 4 * P : (g + 1) * 4 * P],
                            start=True,
                            stop=False,
                        )
                        nc.tensor.matmul(
                            out=w_ps[:],
                            lhsT=ident_b[:],
                            rhs=k_cm[:, cg, g * 4 * P : (g + 1) * 4 * P],
                            start=False,
                            stop=True,
                        )
                        nc.scalar.activation(
                            t1[:, cg, g * 4 * P : (g + 1) * 4 * P],
                            w_ps[:],
                            AFT.Relu,
                            bias=sh1_sb[:, cg : cg + 1],
                            scale=1.0,
                        )

                tap(t, "tap_t1", t1[:, :, :].rearrange("p g n -> p (g n)"))
                # W1 (256->32) + bn2+relu
                t2 = bigpool.tile([CS, NE], BF16, tag="t2")
                for g in range(4):
                    w1_ps = ps_b.tile([CS, 4 * P], F32, tag="pb")
                    for cg in range(2):
                        nc.tensor.matmul(
                            out=w1_ps[:],
                            lhsT=w1_sb[:, cg, :],
                            rhs=t1[:, cg, g * 4 * P : (g + 1) * 4 * P],
                            start=(cg == 0),
                            stop=(cg == 1),
                        )
                    nc.scalar.activation(
                        t2[:, g * 4 * P : (g + 1) * 4 * P],
                        w1_ps[:],
                        AFT.Relu,
                        bias=sh2_sb[:, 0:1],
                        scale=1.0,
                    )

                tap(t, "tap_t2", t2[:, :])
                # per-ns W2 -> point-major logits [128, (ns, cs)], exp
                attn_ps = ps_a.tile([P, NS * CS], F32, tag="pa")
                for ns in range(NS):
                    nc.tensor.matmul(
                        out=attn_ps[:, ns * CS : (ns + 1) * CS],
                        lhsT=t2[:, ns * P : (ns + 1) * P],
                        rhs=w2_sb[:, :],
                        start=True,
                        stop=True,
                    )
                attnE = pool.tile([P, NS * CS], BF16, tag="attnE")
                nc.scalar.activation(attnE[:], attn_ps[:], AFT.Exp)

                tap(t, "tap_attnE", attnE[:, :])
                # softmax denominator: tree-reduce over ns (stride CS)
                scr = pool.tile([P, 12 * CS], BF16, tag="scr")
                v0 = attnE[:, :].rearrange("p (s c) -> p s c", c=CS)
                r1 = scr[:, 0 : 8 * CS].rearrange("p (s c) -> p s c", c=CS)
                nc.vector.tensor_tensor(
                    out=r1, in0=v0[:, 0:16:2, :], in1=v0[:, 1:16:2, :],
                    op=AOP.add,
                )
                r2 = scr[:, 8 * CS : 12 * CS].rearrange(
                    "p (s c) -> p s c", c=CS
                )
                nc.vector.tensor_tensor(
                    out=r2, in0=r1[:, 0:8:2, :], in1=r1[:, 1:8:2, :],
                    op=AOP.add,
                )
                ssum = pool.tile([P, CS], F32, tag="ssum")
                s3 = pool.tile([P, 2 * CS], F32, tag="s3")
                s3v = s3[:, :].rearrange("p (s c) -> p s c", c=CS)
                nc.vector.tensor_tensor(
                    out=s3v, in0=r2[:, 0:4:2, :], in1=r2[:, 1:4:2, :],
                    op=AOP.add,
                )
                nc.vector.tensor_tensor(
                    out=ssum[:].rearrange("p (s c) -> p s c", c=CS),
                    in0=s3v[:, 0:1, :],
                    in1=s3v[:, 1:2, :],
                    op=AOP.add,
                )
                tap(t, "tap_ssum", ssum[:, :])
                rcp = pool.tile([P, CS], F32, tag="rcp")
                nc.vector.reciprocal(rcp[:], ssum[:])
                attn_n = pool.tile([P, NS * CS], BF16, tag="attn_n")
                nc.vector.tensor_tensor(
                    out=attn_n[:].rearrange("p (s c) -> p s c", c=CS),
                    in0=attnE[:].rearrange("p (s c) -> p s c", c=CS),
                    in1=rcp[:]
                    .rearrange("p (o c) -> p o c", o=1)
                    .to_broadcast([P, NS, CS]),
                    op=AOP.mult,
                )

                # ---- vals = v + pe (point-major), product, ns-reduce ----
                vals_sb = bigpool.tile([P, NS * C], BF16, tag="vals_sb")
                for qt in range(8):
                    v_ps = ps_b.tile([P, 2 * C], F32, tag="pb")
                    for j in range(2):
                        ns = qt * 2 + j
                        nc.tensor.matmul(
                            out=v_ps[:, j * C : (j + 1) * C],
                            lhsT=T2_sb[:, ns * P : (ns + 1) * P],
                            rhs=w2v_sb[:, :],
                            start=True,
                            stop=False,
                        )
                        nc.tensor.matmul(
                            out=v_ps[:, j * C : (j + 1) * C],
                            lhsT=ident_b[:],
                            rhs=stag3[:, ns, C : 2 * C],
                            start=False,
                            stop=True,
                        )
                    nc.scalar.copy(
                        vals_sb[:, qt * 2 * C : (qt + 1) * 2 * C], v_ps[:]
                    )

                tap(t, "tap_vals", vals_sb[:, :])
                prod = vals_sb
                nc.vector.tensor_tensor(
                    out=prod[:].rearrange(
                        "p (s g c) -> p s g c", g=S, c=CS
                    ),
                    in0=vals_sb[:].rearrange(
                        "p (s g c) -> p s g c", g=S, c=CS
                    ),
                    in1=attn_n[:]
                    .rearrange("p (s o c) -> p s o c", o=1, c=CS)
                    .to_broadcast([P, NS, S, CS]),
                    op=AOP.mult,
                )
                # tree-reduce over ns: 16 -> 8 -> 4 -> 2 -> 1
                tscr = bigpool.tile([P, 14 * C], BF16, tag="tscr")
                pv = prod[:, :].rearrange("p (s c) -> p s c", c=C)
                u1 = tscr[:, 0 : 8 * C].rearrange("p (s c) -> p s c", c=C)
                nc.vector.tensor_tensor(
                    out=u1, in0=pv[:, 0:16:2, :], in1=pv[:, 1:16:2, :],
                    op=AOP.add,
                )
                u2 = tscr[:, 8 * C : 12 * C].rearrange(
                    "p (s c) -> p s c", c=C
                )
                nc.vector.tensor_tensor(
                    out=u2, in0=u1[:, 0:8:2, :], in1=u1[:, 1:8:2, :],
                    op=AOP.add,
                )
                u3 = tscr[:, 12 * C : 14 * C].rearrange(
                    "p (s c) -> p s c", c=C
                )
                nc.vector.tensor_tensor(
                    out=u3, in0=u2[:, 0:4:2, :], in1=u2[:, 1:4:2, :],
                    op=AOP.add,
                )
                agg = pool.tile([P, C], BF16, tag="agg")
                nc.vector.tensor_tensor(
                    out=agg[:].rearrange("p (s c) -> p s c", c=C),
                    in0=u3[:, 0:1, :],
                    in1=u3[:, 1:2, :],
                    op=AOP.add,
                )

                tap(t, "tap_agg", agg[:, :])
                # residual + rho bn/relu (point-major, params replicated)
                rin = pool.tile([P, C], BF16, tag="rin")
                nc.vector.tensor_tensor(
                    out=rin[:], in0=agg[:], in1=feats_pm[:], op=AOP.add
                )
                nc.vector.tensor_tensor(
                    out=rin[:], in0=rin[:], in1=sclr_sb[:], op=AOP.mult
                )
                nc.vector.tensor_tensor(
                    out=rin[:], in0=rin[:], in1=shfr_sb[:], op=AOP.add
                )
                nc.vector.tensor_scalar_max(rin[:], rin[:], 0.0)

                tap(t, "tap_rin", rin[:, :])
                # final linear: transpose to channel-major, matmul, +r_b
                rT_sb = pool.tile([P, 2, P], BF16, tag="rT_sb")
                for cg in range(2):
                    rT_ps = ps_b.tile([P, P], BF16, tag="pb")
                    nc.tensor.transpose(
                        out=rT_ps[:],
                        in_=rin[:, cg * P : (cg + 1) * P],
                        identity=ident_b[:],
                    )
                    nc.vector.tensor_copy(rT_sb[:, cg, :], rT_ps[:])
                o_ps = ps_a.tile([P, C], F32, tag="pa")
                for cg in range(2):
                    nc.tensor.matmul(
                        out=o_ps[:],
                        lhsT=rT_sb[:, cg, :],
                        rhs=rw2_sb[:, cg, :],
                        start=(cg == 0),
                        stop=(cg == 1),
                    )
                out_sb = pool.tile([P, C], F32, tag="out_sb")
                nc.vector.scalar_tensor_tensor(
                    out=out_sb[:],
                    in0=o_ps[:],
                    scalar=1.0,
                    in1=rb_sb[:],
                    op0=AOP.mult,
                    op1=AOP.add,
                )
                nc.sync.dma_start(out=out_ext[rsl, :], in_=out_sb[:])

    nc.compile()
    return nc


def prep_weights(inputs):
    """Host-side folding of BN/bias into matmul weights."""
    g1, b1, m1, v1 = [inputs["w_bn1"][i] for i in range(4)]
    scale1 = g1 / np.sqrt(v1 + EPS)
    mean_eff = m1 - (
        inputs["bk"] - inputs["bq"] + inputs["p_b2"] + inputs["v_b2"]
    )
    shift1 = b1 - scale1 * mean_eff

    wq_s = (scale1[:, None] * inputs["Wq"]).T  # [cin, cout]
    wq_t = np.stack([_bf(wq_s[0:128]), _bf(wq_s[128:256])])
    wk_s = (scale1[:, None] * inputs["Wk"]).T
    wv = np.asarray(inputs["Wv"]).T
    wkv = np.concatenate([wk_s, wv], axis=1)  # [256, 512]
    wkv_t = np.stack([_bf(wkv[0:128]), _bf(wkv[128:256])])

    gp, bp, mp, vp = [inputs["p_bn"][i] for i in range(4)]
    scale_p = gp / np.sqrt(vp + EPS)
    A1 = scale_p[:, None] * inputs["p_w1"]
    c1 = bp - scale_p * (mp - inputs["p_b1"])
    a1_t = np.zeros((4, 4), np.float32)
    a1_t[:3, :3] = A1.T

    gv, bv_, mv, vv = [inputs["v_bn"][i] for i in range(4)]
    scale_v = (gv / np.sqrt(vv + EPS))[0]
    sv = scale_v * inputs["v_w1"][0, 0]
    bvp = scale_v * (inputs["v_b1"][0] - mv[0]) + bv_[0]

    w2cat_w = np.zeros((4, C), np.float32)
    w2cat_w[0:3] = inputs["p_w2"].T * scale1[None, :]
    w2cat_w[3] = inputs["v_w2"][:, 0] * scale1
    w2cat_v = np.zeros((4, C), np.float32)
    w2cat_v[0:3] = inputs["p_w2"].T
    w2cat_v[3] = inputs["v_w2"][:, 0]

    g2, b2, m2, v2 = [inputs["w_bn2"][i] for i in range(4)]
    scale2 = g2 / np.sqrt(v2 + EPS)
    shift2 = b2 - scale2 * (m2 - inputs["w_b1"])
    w1s = (scale2[:, None] * inputs["w_w1"]).T  # [256, 32]
    w1_t = np.stack([_bf(w1s[0:128]), _bf(w1s[128:256])])
    w2_t = _bf(np.asarray(inputs["w_w2"]).T)

    gr, br, mr, vr = [inputs["r_bn"][i] for i in range(4)]
    scale_r = gr / np.sqrt(vr + EPS)
    mean_r = mr - (inputs["bv"] + inputs["p_b2"] + inputs["v_b2"])
    shift_r = br - scale_r * mean_r
    rw2s = np.asarray(inputs["r_w"]).T
    rw2 = np.stack([_bf(rw2s[0:128]), _bf(rw2s[128:256])])

    viota = np.tile(np.arange(64, dtype=np.float32), NS)[None, :]

    return {
        "wq_t": wq_t,
        "wkv_t": wkv_t,
        "w2cat_w": _bf(w2cat_w),
        "w2cat_v": _bf(w2cat_v),
        "w1_t": w1_t,
        "w2_t": w2_t,
        "rw2": rw2,
        "a1_t": _f32(a1_t),
        "shift1_c": _f32(np.asarray(shift1).reshape(2, 128).T),
        "shift2_c": _f32(np.asarray(shift2)[:, None]),
        "svbv_c": _f32(np.array([[sv, bvp]])),
        "c1_c": _f32(np.pad(np.asarray(c1, np.float64), (0, 1))[None, :]),
        "scaler_c": _bf(scale_r[None, :]),
        "shiftr_c": _bf(shift_r[None, :]),
        "rb_c": _f32(np.asarray(inputs["r_b"])[None, :]),
        "viota_c": np.ascontiguousarray(viota),
    }


def wrap_fat_idx(idxv_shard):
    """Per-tile wrapped int16 layout of idx_v//64 for dma_gather,
    split into two 1024-idx halves (neighbors 0-7 / 8-15)."""
    npc = idxv_shard.shape[0]
    nt = npc // P
    fat = (np.asarray(idxv_shard, np.int64) // 64).astype(np.int16)
    out = np.empty((nt, 2, P, NE // 32), np.int16)
    for t in range(nt):
        flat = fat[t * P : (t + 1) * P].T.ravel()  # i = ns*128 + n
        for h in range(2):
            half = flat[h * (NE // 2) : (h + 1) * (NE // 2)]
            wrap = half.reshape(NE // 32, 16).T  # [16, 64]
            out[t, h] = np.tile(wrap, (8, 1))
    return np.ascontiguousarray(out)


_PROGRAM_CACHE = {}


def run(inputs, n_total, debug_taps=False, **spmd_kwargs):
    npc = n_total // NCORES
    key = (n_total, debug_taps)
    if key not in _PROGRAM_CACHE:
        _PROGRAM_CACHE[key] = build_program(n_total, debug_taps)
    nc = _PROGRAM_CACHE[key]

    wd = prep_weights(inputs)
    in_maps = []
    for c in range(NCORES):
        sl = slice(c * npc, (c + 1) * npc)
        m = {
            "feats_sh": _f32(inputs["feats"][sl]),
            "xyz_sh": _f32(inputs["xyz"][sl]),
            "vel_sh": _f32(inputs["velocities"][sl]),
            "idx_sh": np.ascontiguousarray(inputs["idx"][sl], np.int32),
            "idxvf_sh": wrap_fat_idx(inputs["idx_v"][sl]),
            "vsel_sh": _f32(np.asarray(inputs["idx_v"][sl]) % 64),
        }
        m.update(wd)
        in_maps.append(m)

    res = run_bass_kernel_spmd(
        nc, in_maps, core_ids=list(range(NCORES)), **spmd_kwargs
    )
    out = np.concatenate([r["out"] for r in res.results], axis=0)
    return out, res


def kernel(**inputs):
    inputs = {k: np.asarray(v) for k, v in inputs.items()}
    n_total = inputs["feats"].shape[0]
    out, _ = run(inputs, n_total)
    return np.ascontiguousarray(out, dtype=np.float32)

